# revision 18
# baseline (speedup 1.0000x reference)
"""GCN (3-layer GraphConv, norm='right') Trainium2 Bass kernel — 8-core SPMD.

Sharding: nodes are split into 8 contiguous shards of R=6272 rows (padded to
50176 = 392 blocks of 128). Core c owns rows [cR,(c+1)R): it holds that shard
of the (projected) feature table and processes every edge whose SRC lies in
its shard, so all gathers are local. Each core accumulates partial aggregates
for ALL dst blocks (one-hot S-matmuls into PSUM, inv_deg folded into S, done
transposed: stationary=msg, moving=S, so partials are feature-major and no
PE transposes are ever needed); ReduceScatter(add) over the 8 cores then
hands core c the complete aggregate for its own 49 dst blocks. The epilogue
applies bias+ReLU and the next layer's projection, writing the next local
table. Layer-3's table is pre-projected by W2 so its aggregation directly
yields logits; b2/8 is added on every core so the RS sum restores b2.

Edge layout (straddle): blocks are processed j-major (round j = blocks 49c+j
for all c) and each block's edge region is sized max_c cnt[c,b] — regions are
packed contiguously WITHOUT per-block 128-alignment, so a 128-edge gather
tile may straddle adjacent block regions; each (tile, block-overlap) gets its
own one-hot S matmul (pair). This keeps gather padding at ~9% instead of
~50%. Gathered rows cost one DMA descriptor each (~1.42ns effective), making
rows-gathered the dominant term; everything else (S-builds on DVE, fp16
matmuls on PE, copies on ACT, partial writes) overlaps under it.

The per-layer partial table is split into three j-ranges so the first two
ReduceScatters (and their epilogues) overlap the remaining aggregation; only
the last small RS sits on the layer tail. All small DMAs are batched 8
blocks at a time (HWDGE fixed cost ~625ns each); PSUM holds 8 aggregation
targets per 2-bank tile so one ACT copy moves a whole round. Region sizes
are equalized across cores (max over cores) so all 8 cores run the identical
program (SPMD); padding edges carry slot=999 and contribute exactly zero.
"""
import numpy as np

import concourse.bass as bass
import concourse.tile as tile
from concourse import bacc, mybir
from concourse.bass_utils import run_bass_kernel_spmd

N_NODES = 50000
N_EDGES = 800000
IN_FEATS, F, N_CLASSES = 128, 64, 40
NCORES = 8
NBLK = 392                      # dst blocks of 128 rows
NROWS = NBLK * 128              # 50176
BPC = NBLK // NCORES            # 49 blocks per core
R = BPC * 128                   # 6272 rows per core
CH = 40                         # gather chunk size in tiles (128 idxs each)
JS = (0, 22, 38, 46, BPC)       # ReduceScatter split points (rounds)

_cache = {}


def _prep(features, src, dst, W0, b0, W1, b1, W2, b2):
    deg = np.bincount(dst, minlength=N_NODES).astype(np.float32)
    invd = (1.0 / np.maximum(deg, 1.0)).astype(np.float32)

    core = src // R
    blk = dst // 128
    cnt = np.zeros((NCORES, NBLK), dtype=np.int64)
    np.add.at(cnt, (core, blk), 1)
    nb = np.maximum(cnt.max(axis=0), 1)              # region size per block

    # processing order: j-major
    order = np.array([BPC * c + j for j in range(BPC) for c in range(NCORES)])
    estart = np.zeros(NBLK, dtype=np.int64)
    estart[order] = np.concatenate([[0], np.cumsum(nb[order])[:-1]])
    E_tot = int(nb.sum())
    T = (E_tot + 127) // 128

    t_lo = estart // 128
    t_hi = (estart + nb - 1) // 128
    npairs = (t_hi - t_lo + 1).astype(np.int64)
    pair_base = np.zeros(NBLK, dtype=np.int64)
    pair_base[order] = np.concatenate([[0], np.cumsum(npairs[order])[:-1]])
    NP = int(npairs.sum())
    pair_tile = np.zeros(NP, dtype=np.int64)
    for b in range(NBLK):
        pair_tile[pair_base[b]:pair_base[b] + npairs[b]] = np.arange(
            t_lo[b], t_hi[b] + 1)

    idx_d, slotp_d, invd_d = {}, {}, {}
    for c in range(NCORES):
        m = core == c
        s_c = src[m] - c * R
        d_c = dst[m]
        b_c = blk[m]
        o = np.argsort(b_c, kind="stable")
        s_c, d_c, b_c = s_c[o], d_c[o], b_c[o]
        cc = cnt[c]
        starts = np.concatenate([[0], np.cumsum(cc)[:-1]])
        rank = np.arange(len(s_c)) - np.repeat(starts, cc)
        pos = estart[b_c] + rank
        tile_e = pos // 128
        lane = pos % 128

        idx_pad = np.zeros(T * 128, dtype=np.int16)
        invd_pad = np.zeros(T * 128, dtype=np.float32)
        idx_pad[pos] = s_c.astype(np.int16)
        invd_pad[pos] = invd[d_c]
        slotp = np.full((128, NP), 999.0, dtype=np.float32)
        slotp[lane, pair_base[b_c] + tile_e - t_lo[b_c]] = (
            d_c % 128).astype(np.float32)

        idx_d[c] = np.tile(idx_pad.reshape(-1, 16).T, (8, 1)).copy()  # [128,T*8]
        invd_d[c] = np.ascontiguousarray(invd_pad.reshape(T, 128).T)  # [128,T]
        slotp_d[c] = slotp

    W2p = np.zeros((F, F), dtype=np.float16)
    W2p[:, :N_CLASSES] = W2[:, :N_CLASSES].astype(np.float16)
    b2p = np.zeros((F, 1), dtype=np.float32)
    b2p[:min(len(b2), F), 0] = np.asarray(b2).reshape(-1)[:F] / NCORES

    xpad = np.zeros((NROWS, IN_FEATS), dtype=np.float32)
    xpad[:N_NODES] = features
    iota16 = np.tile(np.arange(128, dtype=np.float16), (128, 1))

    in_maps = []
    for c in range(NCORES):
        in_maps.append({
            "xT": np.ascontiguousarray(xpad[c * R:(c + 1) * R].T).astype(np.float16),
            "idx": idx_d[c], "slotp": slotp_d[c], "invdm": invd_d[c],
            "W0": np.ascontiguousarray(W0.astype(np.float16)),
            "W1": np.ascontiguousarray(W1.astype(np.float16)),
            "W2p": W2p,
            "b0": np.asarray(b0, dtype=np.float32).reshape(F, 1),
            "b1": np.asarray(b1, dtype=np.float32).reshape(F, 1),
            "b2p": b2p,
            "iota16": iota16,
        })
    sched = (tuple(npairs.tolist()), tuple(pair_tile.tolist()),
             tuple(pair_base.tolist()), T, NP)
    return in_maps, sched


def _build(sched):
    npairs, pair_tile, pair_base, T, NP = sched
    # last tile index (+1) consumed by the end of each round
    round_end = np.zeros(BPC, dtype=np.int64)
    hi = 0
    for j in range(BPC):
        for c in range(NCORES):
            b = BPC * c + j
            p_last = pair_base[b] + npairs[b] - 1
            hi = max(hi, pair_tile[p_last] + 1)
        round_end[j] = hi

    nc = bacc.Bacc("TRN2", num_devices=NCORES, dynamic_dma_scratch_size=65536)
    dt = mybir.dt.float32
    f16 = mybir.dt.float16

    xT_in = nc.dram_tensor("xT", [IN_FEATS, R], f16, kind="ExternalInput")
    idx_in = nc.dram_tensor("idx", [128, T * 8], mybir.dt.int16, kind="ExternalInput")
    slot_in = nc.dram_tensor("slotp", [128, NP], dt, kind="ExternalInput")
    invd_in = nc.dram_tensor("invdm", [128, T], dt, kind="ExternalInput")
    W0_in = nc.dram_tensor("W0", [IN_FEATS, F], f16, kind="ExternalInput")
    W1_in = nc.dram_tensor("W1", [F, F], f16, kind="ExternalInput")
    W2_in = nc.dram_tensor("W2p", [F, F], f16, kind="ExternalInput")
    b0_in = nc.dram_tensor("b0", [F, 1], dt, kind="ExternalInput")
    b1_in = nc.dram_tensor("b1", [F, 1], dt, kind="ExternalInput")
    b2_in = nc.dram_tensor("b2p", [F, 1], dt, kind="ExternalInput")
    iota_in = nc.dram_tensor("iota16", [128, 128], f16, kind="ExternalInput")
    out = nc.dram_tensor("out", [BPC, N_CLASSES, 128], dt, kind="ExternalOutput")

    NSEG = len(JS) - 1

    with tile.TileContext(nc) as tc:
        with tc.tile_pool(name="const", bufs=1) as cp, \
             tc.tile_pool(name="dram", bufs=1, space="DRAM") as dram, \
             tc.tile_pool(name="msg", bufs=6) as mp, \
             tc.tile_pool(name="msg16", bufs=6) as m16p, \
             tc.tile_pool(name="stl", bufs=10) as sp, \
             tc.tile_pool(name="xbp", bufs=2) as xbp, \
             tc.tile_pool(name="po", bufs=3) as pop, \
             tc.tile_pool(name="ep", bufs=2) as epp, \
             tc.tile_pool(name="agg", bufs=2, space="PSUM") as pp, \
             tc.tile_pool(name="eps", bufs=2, space="PSUM") as pp2:

            W0_t = cp.tile([IN_FEATS, F], f16)
            nc.sync.dma_start(W0_t[:], W0_in[:])
            W1_t = cp.tile([F, F], f16)
            nc.sync.dma_start(W1_t[:], W1_in[:])
            W2_t = cp.tile([F, F], f16)
            nc.sync.dma_start(W2_t[:], W2_in[:])
            b0_t = cp.tile([F, 1], dt)
            nc.sync.dma_start(b0_t[:], b0_in[:])
            b1_t = cp.tile([F, 1], dt)
            nc.sync.dma_start(b1_t[:], b1_in[:])
            b2_t = cp.tile([F, 1], dt)
            nc.sync.dma_start(b2_t[:], b2_in[:])
            iota_t = cp.tile([128, 128], f16)
            nc.sync.dma_start(iota_t[:], iota_in[:])
            idx_t = cp.tile([128, T * 8], mybir.dt.int16)
            nc.sync.dma_start(idx_t[:], idx_in[:])
            slot_t = cp.tile([128, NP], dt)
            nc.sync.dma_start(slot_t[:], slot_in[:])
            invd_t = cp.tile([128, T], dt)
            nc.sync.dma_start(invd_t[:], invd_in[:])

            # partial tables are pair-packed [c, j//2, w, 2, 128] so DMA
            # descriptors are 512B (256B descriptors pay a 2x latency penalty)
            tbl = [dram.tile([R, F], dt, tag=f"t{l}", name=f"t{l}") for l in range(3)]
            parts, rss = [], []
            for l in range(3):
                w = F if l < 2 else N_CLASSES
                parts.append([dram.tile(
                    [NCORES, (JS[s + 1] - JS[s] + 1) // 2, w, 2, 128], f16,
                    tag=f"p{l}s{s}", name=f"p{l}s{s}") for s in range(NSEG)])
                rss.append([dram.tile(
                    [(JS[s + 1] - JS[s] + 1) // 2, w, 2, 128], f16,
                    tag=f"rs{l}s{s}", name=f"rs{l}s{s}") for s in range(NSEG)])

            # ---- initial projection: t0 = X_c @ W0, batched 8 blocks ----
            for k0 in range(0, BPC, 8):
                g = min(8, BPC - k0)
                xb = xbp.tile([IN_FEATS, 8, 128], f16, tag="xb")
                nc.sync.dma_start(xb[:, 0:g, :].rearrange("f g s -> f (g s)"),
                                  xT_in[:, k0 * 128:(k0 + g) * 128])
                pj = pp2.tile([128, 8, F], dt, tag="pj8")
                for i in range(g):
                    nc.tensor.matmul(pj[:, i, :], xb[:, i, :], W0_t[:],
                                     start=True, stop=True)
                ys = epp.tile([128, 8, F], dt, tag="ys8")
                nc.scalar.activation(ys[:, 0:g, :], pj[:, 0:g, :],
                                     mybir.ActivationFunctionType.Identity)
                nc.sync.dma_start(
                    tbl[0][k0 * 128:(k0 + g) * 128, :].rearrange(
                        "(g p) f -> p g f", g=g), ys[:, 0:g, :])

            # ---- layers ----
            def epilogue(l, seg):
                jlo, jhi = JS[seg], JS[seg + 1]
                rs = rss[l][seg]
                if l < 2:
                    bias = b0_t if l == 0 else b1_t
                    Wn = W1_t if l == 0 else W2_t
                    for k0 in range(jlo, jhi, 8):
                        g = min(8, jhi - k0)
                        npr = (g + 1) // 2
                        p0 = (k0 - jlo) // 2
                        a16 = epp.tile([F, 8, 128], f16, tag="a16")
                        nc.sync.dma_start(
                            a16[:, 0:2 * npr, :].rearrange("f (p j) s -> f p j s",
                                                           j=2),
                            rs[p0:p0 + npr].rearrange("p f j s -> f p j s"))
                        hT = epp.tile([F, 8, 128], f16, tag="hT")
                        nc.scalar.activation(hT[:, 0:g, :], a16[:, 0:g, :],
                                             mybir.ActivationFunctionType.Relu,
                                             bias=bias[:, 0:1])
                        y8 = pp2.tile([128, 8, F], dt, tag="pj8")
                        for i in range(g):
                            nc.tensor.matmul(y8[:, i, :], hT[:, i, :], Wn[:],
                                             start=True, stop=True)
                        ys = epp.tile([128, 8, F], dt, tag="ys8")
                        nc.scalar.activation(ys[:, 0:g, :], y8[:, 0:g, :],
                                             mybir.ActivationFunctionType.Identity)
                        nc.sync.dma_start(
                            tbl[l + 1][k0 * 128:(k0 + g) * 128, :].rearrange(
                                "(g p) f -> p g f", g=g), ys[:, 0:g, :])
                else:
                    for k0 in range(jlo, jhi, 8):
                        g = min(8, jhi - k0)
                        npr = (g + 1) // 2
                        p0 = (k0 - jlo) // 2
                        o16 = epp.tile([N_CLASSES, 8, 128], f16, tag="o16")
                        nc.sync.dma_start(
                            o16[:, 0:2 * npr, :].rearrange("f (p j) s -> f p j s",
                                                           j=2),
                            rs[p0:p0 + npr].rearrange("p f j s -> f p j s"))
                        of = epp.tile([N_CLASSES, 8, 128], dt, tag="of")
                        nc.scalar.activation(of[:, 0:g, :], o16[:, 0:g, :],
                                             mybir.ActivationFunctionType.Identity)
                        nc.sync.dma_start(
                            out[k0:k0 + g].rearrange("g f s -> f g s"),
                            of[:, 0:g, :])

            for l in range(3):
                table = tbl[l]
                ph_f = F if l < 2 else N_CLASSES
                msgs = []

                def emit_chunk():
                    ch = len(msgs)
                    nt = min(CH, T - ch * CH)
                    msg = mp.tile([128, CH, F], dt, tag="msg")
                    nc.gpsimd.dma_gather(
                        msg[:, 0:nt, :], table[:],
                        idx_t[:, ch * CH * 8: ch * CH * 8 + nt * 8],
                        num_idxs=nt * 128, num_idxs_reg=nt * 128,
                        elem_size=F, single_packet=False)
                    msg16 = m16p.tile([128, CH, F], f16, tag="msg16")
                    nc.scalar.activation(msg16[:, 0:nt, :], msg[:, 0:nt, :],
                                         mybir.ActivationFunctionType.Identity)
                    msgs.append(msg16)

                po2 = None
                tile_ptr = 0
                for j in range(BPC):
                    for s in range(NSEG):
                        if j == JS[s + 1]:      # segment s complete -> RS
                            nc.gpsimd.collective_compute(
                                "ReduceScatter", mybir.AluOpType.add,
                                replica_groups=[list(range(NCORES))],
                                ins=[parts[l][s][:]], outs=[rss[l][s][:]])
                            # epilogue of the segment BEFORE last: its RS is
                            # long done, so its DMAs complete well before the
                            # next RS's inputs and never gate it (the hw DMA
                            # queue semaphores are count-based).
                            if s >= 1:
                                epilogue(l, s - 1)
                    seg = next(s for s in range(NSEG) if JS[s] <= j < JS[s + 1])
                    agg = pp.tile([F, 8, 128], dt, tag="agg")
                    for c in range(NCORES):
                        b = BPC * c + j
                        np_b = int(npairs[b])
                        for i in range(np_b):
                            p = int(pair_base[b]) + i
                            t = int(pair_tile[p])
                            tile_ptr = max(tile_ptr, t + 1)
                            while t >= len(msgs) * CH:
                                emit_chunk()
                            S = sp.tile([128, 128], f16, tag="S")
                            nc.vector.tensor_scalar(
                                S[:], iota_t[:], slot_t[:, p:p + 1],
                                invd_t[:, t:t + 1],
                                mybir.AluOpType.is_equal, mybir.AluOpType.mult)
                            nc.tensor.matmul(
                                agg[:, c, :], msgs[t // CH][:, t % CH, :], S[:],
                                start=(i == 0), stop=(i == np_b - 1))
                    q = (j - JS[seg]) % 2
                    if q == 0:
                        po2 = pop.tile([ph_f, NCORES, 2, 128], f16, tag="po")
                    if l < 2:
                        nc.scalar.activation(po2[:, :, q, :], agg[:],
                                             mybir.ActivationFunctionType.Identity)
                    else:
                        nc.scalar.activation(po2[:, :, q, :], agg[0:N_CLASSES, :, :],
                                             mybir.ActivationFunctionType.Identity,
                                             bias=b2_t[0:N_CLASSES, 0:1])
                    if q == 1 or j == JS[seg + 1] - 1:
                        jp = (j - JS[seg]) // 2
                        nc.sync.dma_start(
                            parts[l][seg][:, jp, :, 0:q + 1, :].rearrange(
                                "g f j s -> f g j s"),
                            po2[:, :, 0:q + 1, :])

                # final segment RS + remaining epilogues
                nc.gpsimd.collective_compute(
                    "ReduceScatter", mybir.AluOpType.add,
                    replica_groups=[list(range(NCORES))],
                    ins=[parts[l][NSEG - 1][:]], outs=[rss[l][NSEG - 1][:]])
                epilogue(l, NSEG - 2)
                epilogue(l, NSEG - 1)

    nc.compile()
    return nc


def kernel(features, src, dst, W0, b0, W1, b1, W2, b2):
    features = np.asarray(features, dtype=np.float32)
    src = np.asarray(src).astype(np.int64)
    dst = np.asarray(dst).astype(np.int64)
    in_maps, sched = _prep(features, src, dst,
                           np.asarray(W0), np.asarray(b0), np.asarray(W1),
                           np.asarray(b1), np.asarray(W2), np.asarray(b2))
    if _cache.get("key") != sched:
        _cache["nc"] = _build(sched)
        _cache["key"] = sched
    nc = _cache["nc"]
    res = run_bass_kernel_spmd(nc, in_maps, core_ids=list(range(NCORES)))
    shards = []
    for c in range(NCORES):
        o = res.results[c]["out"]                      # [BPC, 40, 128]
        shards.append(o.transpose(0, 2, 1).reshape(R, N_CLASSES))
    full = np.concatenate(shards, axis=0)
    return np.ascontiguousarray(full[:N_NODES])
